# Initial kernel scaffold
#
"""Trainium2 Bass kernel for GQA attention (dense_transformer).

Sharding (8 cores): core c = (batch b = c//4, tp-rank g = c%4).
Each core computes q/k/v projections + RoPE + causal attention + partial
output projection for heads {g, g+4, g+8, g+12} (which all map to kv head
g under the reference's jnp.tile repeat), then a 4-way ReduceScatter over
the batch group combines the output projection partials; each core ends
up with a disjoint quarter of each 512-row i-chunk of the output.

Layout choices:
  - All matmuls run in float32r (tf32-class, full PE rate at N>=256).
  - Scores are computed transposed (scores^T[j, i]) so softmax probs feed
    the P@V matmul directly as the moving operand; the softmax denominator
    comes from an extra all-ones stationary matmul into a separate PSUM
    bank (partition-broadcast for free).
  - No max-subtraction in softmax: scores are O(6) sigma for this problem
    scale, exp() stays well inside fp32 range.  The additive mask is
    applied as exp(mask) multiplicative block patterns; all-zero blocks
    are skipped entirely (causality), all-pass blocks skip the multiply.
  - x is transposed on host (d-major) so every matmul contracts along
    partitions without any on-device transposes (v needs 128x128 PE
    transposes only).
"""

import sys

sys.path.insert(0, "/opt/trn_rl_repo")

import numpy as np

# ---------------------------------------------------------------- constants
B = 2
S = 2048
DIM = 2048
NH = 16
NKV = 4
HD = 128          # head dim == partition count
P = 128
CH = 512          # i-chunk columns (moving dim)
NQC = S // CH     # 4 i-chunks
NJT = S // P      # 16 j-tiles
DT = DIM // P     # 16 d-tiles (contraction)
HPC = NH // 4     # 4 heads per core
NOC = DIM // CH   # 4 output column chunks
N_CORES = 8
GROUPS = [[0, 1, 2, 3], [4, 5, 6, 7]]
SCALE = 1.0 / float(np.sqrt(HD))

_prog_cache: dict = {}


def _mask_schedule(mask):
    """Per (i-chunk, j-tile) block status from the additive mask.

    Returns (sched, patterns): sched[qc] = tuple of (jt, pat_idx|None) for
    non-skipped j-tiles; patterns = list of [P, CH] f32 multiplicative
    exp(mask) blocks (transposed to [j, i] layout).
    """
    m = np.asarray(mask, dtype=np.float32).reshape(S, S)
    pats = {}
    plist = []
    sched = []
    for qc in range(NQC):
        row = []
        for jt in range(NJT):
            blk = m[qc * CH:(qc + 1) * CH, jt * P:(jt + 1) * P]
            em = np.exp(blk.astype(np.float64)).astype(np.float32)
            if np.all(em == 1.0):
                row.append((jt, None))
            elif np.all(em == 0.0):
                continue
            else:
                pt = np.ascontiguousarray(em.T)  # [j=128, i=512]
                key = pt.tobytes()
                if key not in pats:
                    pats[key] = len(plist)
                    plist.append(pt)
                row.append((jt, pats[key]))
        if not row:
            raise ValueError(
                f"i-chunk {qc} is fully masked; softmax would be undefined"
            )
        sched.append(tuple(row))
    return tuple(sched), plist


def _build_program(sched, n_pat):
    import concourse.bacc as bacc
    import concourse.mybir as mybir
    import concourse.tile as tile

    F32 = mybir.dt.float32
    F32R = mybir.dt.float32r
    AF = mybir.ActivationFunctionType

    nc = bacc.Bacc(None, target_bir_lowering=False, num_devices=N_CORES)

    xt = nc.declare_dram_parameter("xt", [DIM, S], F32R, isOutput=False)
    wqs = nc.declare_dram_parameter("wqs", [DIM, HPC * HD], F32R, isOutput=False)
    wks = nc.declare_dram_parameter("wks", [DIM, HD], F32R, isOutput=False)
    wvs = nc.declare_dram_parameter("wvs", [DIM, HD], F32R, isOutput=False)
    wos = nc.declare_dram_parameter("wos", [HPC * HD, DIM], F32R, isOutput=False)
    cosf = nc.declare_dram_parameter("cosf", [HD, S], F32, isOutput=False)
    sinf = nc.declare_dram_parameter("sinf", [HD, S], F32, isOutput=False)
    onesm = nc.declare_dram_parameter("onesm", [P, P], F32R, isOutput=False)
    ident = nc.declare_dram_parameter("ident", [P, P], F32R, isOutput=False)
    dpat = nc.declare_dram_parameter("dpat", [max(n_pat, 1), P, CH], F32, isOutput=False)
    out = nc.declare_dram_parameter("out", [NQC, CH // 4, DIM], F32, isOutput=True)

    o_part = nc.dram_tensor("o_part", [S, DIM], F32)
    rs_out = nc.dram_tensor("rs_out", [NQC, CH // 4, DIM], F32)

    shuffle_mask = [i ^ 1 for i in range(32)]

    with tile.TileContext(nc) as tc:
        with (
            tc.tile_pool(name="const", bufs=1) as constp,
            tc.tile_pool(name="qT", bufs=HPC) as qTp,
            tc.tile_pool(name="kT", bufs=1) as kTp,
            tc.tile_pool(name="vsb", bufs=NJT) as vsbp,
            tc.tile_pool(name="ohT", bufs=HPC) as ohTp,
            tc.tile_pool(name="tmp", bufs=6) as tmpp,
        ):
            cos_sb = constp.tile([HD, S], F32, tag="cos")
            sin_sb = constp.tile([HD, S], F32, tag="sin")
            ones_sb = constp.tile([P, P], F32R, tag="ones")
            id_sb = constp.tile([P, P], F32R, tag="ident")
            dpat_sb = constp.tile([P, max(n_pat, 1) * CH], F32, tag="dpat")
            nc.sync.dma_start(cos_sb[:], cosf[:])
            nc.sync.dma_start(sin_sb[:], sinf[:])
            nc.sync.dma_start(ones_sb[:], onesm[:])
            nc.sync.dma_start(id_sb[:], ident[:])
            nc.sync.dma_start(
                dpat_sb[:].rearrange("p (n c) -> p n c", c=CH),
                dpat[:].rearrange("n p c -> p n c"),
            )

            qT = [qTp.tile([HD, S], F32R, tag="qT") for _ in range(HPC)]
            kT = kTp.tile([HD, S], F32R, tag="kT")
            vsb = [vsbp.tile([P, HD], F32R, tag="vsb") for _ in range(NJT)]
            ohT = [ohTp.tile([HD, S], F32R, tag="ohT") for _ in range(HPC)]

            # ---------------- phase 1: projections + RoPE -----------------
            with (
                tc.tile_pool(name="wq_sb", bufs=HPC) as wqp,
                tc.tile_pool(name="wkv_sb", bufs=2) as wkvp,
                tc.tile_pool(name="xtp", bufs=4) as xtp,
                tc.tile_pool(name="vt_sb", bufs=2) as vtsbp,
                tc.tile_pool(name="proj_ps", bufs=6, space="PSUM") as proj_ps,
                tc.tile_pool(name="vt_ps", bufs=2, space="PSUM") as vt_ps,
            ):
                wq_sb = []
                for hl in range(HPC):
                    t = wqp.tile([P, DT * HD], F32R, tag="wq")
                    nc.sync.dma_start(
                        t[:].rearrange("p (t m) -> p t m", m=HD),
                        wqs[:, hl * HD:(hl + 1) * HD].rearrange(
                            "(t p) m -> p t m", p=P
                        ),
                    )
                    wq_sb.append(t)
                wk_sb = wkvp.tile([P, DT * HD], F32R, tag="wkv")
                nc.sync.dma_start(
                    wk_sb[:].rearrange("p (t m) -> p t m", m=HD),
                    wks[:].rearrange("(t p) m -> p t m", p=P),
                )
                wv_sb = wkvp.tile([P, DT * HD], F32R, tag="wkv")
                nc.sync.dma_start(
                    wv_sb[:].rearrange("p (t m) -> p t m", m=HD),
                    wvs[:].rearrange("(t p) m -> p t m", p=P),
                )

                for qc in range(NQC):
                    csl = slice(qc * CH, (qc + 1) * CH)
                    ps_q = [proj_ps.tile([P, CH], F32, tag="pps") for _ in range(HPC)]
                    ps_k = proj_ps.tile([P, CH], F32, tag="pps")
                    ps_v = proj_ps.tile([P, CH], F32, tag="pps")
                    for d in range(DT):
                        xtile = xtp.tile([P, CH], F32R, tag="xt")
                        nc.sync.dma_start(
                            xtile[:], xt[d * P:(d + 1) * P, csl]
                        )
                        dsl = slice(d * HD, (d + 1) * HD)
                        st = (d == 0)
                        sp = (d == DT - 1)
                        for hl in range(HPC):
                            nc.tensor.matmul(
                                ps_q[hl][:], wq_sb[hl][:, dsl], xtile[:],
                                start=st, stop=sp,
                            )
                        nc.tensor.matmul(
                            ps_k[:], wk_sb[:, dsl], xtile[:], start=st, stop=sp
                        )
                        nc.tensor.matmul(
                            ps_v[:], wv_sb[:, dsl], xtile[:], start=st, stop=sp
                        )
                    # RoPE on q heads and k
                    for src, dst in [(ps_q[hl], qT[hl]) for hl in range(HPC)] + [
                        (ps_k, kT)
                    ]:
                        swp = tmpp.tile([P, CH], F32, tag="tmp")
                        nc.vector.stream_shuffle(swp[:], src[:], shuffle_mask)
                        tcos = tmpp.tile([P, CH], F32, tag="tmp")
                        nc.vector.tensor_mul(tcos[:], src[:], cos_sb[:, csl])
                        tsin = tmpp.tile([P, CH], F32, tag="tmp")
                        nc.vector.tensor_mul(tsin[:], swp[:], sin_sb[:, csl])
                        nc.vector.tensor_add(dst[:, csl], tcos[:], tsin[:])
                    # v: copy out and transpose into [j, c] tiles
                    vt = vtsbp.tile([P, CH], F32R, tag="vt")
                    nc.scalar.activation(vt[:], ps_v[:], AF.Copy)
                    for jl in range(CH // P):
                        tps = vt_ps.tile([P, P], F32, tag="vtps")
                        nc.tensor.transpose(
                            tps[:], vt[:, jl * P:(jl + 1) * P], id_sb[:]
                        )
                        nc.scalar.activation(
                            vsb[qc * (CH // P) + jl][:], tps[:], AF.Copy
                        )

            # ---------------- phase 2: attention --------------------------
            with (
                tc.tile_pool(name="probs", bufs=3) as probsp,
                tc.tile_pool(name="s_ps", bufs=2, space="PSUM") as s_ps,
                tc.tile_pool(name="av_ps", bufs=2, space="PSUM") as av_ps,
                tc.tile_pool(name="den_ps", bufs=2, space="PSUM") as den_ps,
            ):
                for qc in range(NQC):
                    csl = slice(qc * CH, (qc + 1) * CH)
                    acts = sched[qc]
                    nact = len(acts)
                    for hl in range(HPC):
                        ps_av = av_ps.tile([HD, CH], F32, tag="av")
                        ps_den = den_ps.tile([P, CH], F32, tag="den")
                        for idx, (jt, pidx) in enumerate(acts):
                            ps_s = s_ps.tile([P, CH], F32, tag="s")
                            nc.tensor.matmul(
                                ps_s[:],
                                kT[:, jt * P:(jt + 1) * P],
                                qT[hl][:, csl],
                                start=True, stop=True,
                            )
                            pr = probsp.tile([P, CH], F32R, tag="pr")
                            nc.scalar.activation(
                                pr[:], ps_s[:], AF.Exp, scale=SCALE
                            )
                            if pidx is not None:
                                nc.vector.tensor_mul(
                                    pr[:], pr[:],
                                    dpat_sb[:, pidx * CH:(pidx + 1) * CH],
                                )
                            nc.tensor.matmul(
                                ps_av[:], vsb[jt][:], pr[:],
                                start=(idx == 0), stop=(idx == nact - 1),
                            )
                            nc.tensor.matmul(
                                ps_den[:], ones_sb[:], pr[:],
                                start=(idx == 0), stop=(idx == nact - 1),
                            )
                        inv = tmpp.tile([P, CH], F32, tag="tmp")
                        nc.vector.reciprocal(inv[:], ps_den[:])
                        nc.vector.tensor_mul(ohT[hl][:, csl], ps_av[:], inv[:])

            # ---------------- phase 3: output projection + RS -------------
            with (
                tc.tile_pool(name="wo_sb", bufs=HPC) as wop,
                tc.tile_pool(name="o_sb", bufs=3) as osbp,
                tc.tile_pool(name="o_ps", bufs=3, space="PSUM") as o_ps,
            ):
                wo_sb = []
                for hl in range(HPC):
                    t = wop.tile([HD, DIM], F32R, tag="wo")
                    nc.sync.dma_start(t[:], wos[hl * HD:(hl + 1) * HD, :])
                    wo_sb.append(t)
                for qc in range(NQC):
                    for it in range(CH // P):
                        ti = qc * (CH // P) + it
                        isl = slice(ti * P, (ti + 1) * P)
                        for oc in range(NOC):
                            osl = slice(oc * CH, (oc + 1) * CH)
                            ps_o = o_ps.tile([P, CH], F32, tag="o")
                            for hl in range(HPC):
                                nc.tensor.matmul(
                                    ps_o[:],
                                    ohT[hl][:, isl],
                                    wo_sb[hl][:, osl],
                                    start=(hl == 0), stop=(hl == HPC - 1),
                                )
                            ob = osbp.tile([P, CH], F32, tag="ob")
                            nc.scalar.activation(ob[:], ps_o[:], AF.Copy)
                            nc.sync.dma_start(o_part[isl, osl], ob[:])
                    nc.gpsimd.collective_compute(
                        "ReduceScatter",
                        mybir.AluOpType.add,
                        replica_groups=GROUPS,
                        ins=[o_part[qc * CH:(qc + 1) * CH, :]],
                        outs=[rs_out[qc]],
                    )
                nc.sync.dma_start(out[:], rs_out[:])

    nc.finalize()
    return nc


def _get_program(sched, n_pat):
    key = (sched, n_pat)
    if key not in _prog_cache:
        _prog_cache[key] = _build_program(sched, n_pat)
    return _prog_cache[key]


def kernel(x, wq, wk, wv, wo, freqs_cos, freqs_sin, mask, start_pos=0, **_kw):
    from concourse.bass_utils import run_bass_kernel_spmd

    x = np.asarray(x, dtype=np.float32)
    wq = np.asarray(wq, dtype=np.float32)
    wk = np.asarray(wk, dtype=np.float32)
    wv = np.asarray(wv, dtype=np.float32)
    wo = np.asarray(wo, dtype=np.float32)
    fc = np.asarray(freqs_cos, dtype=np.float32)
    fs = np.asarray(freqs_sin, dtype=np.float32)

    sched, plist = _mask_schedule(mask)
    nc = _get_program(sched, len(plist))

    # RoPE tables expanded to head-dim channels (sin sign-interleaved so the
    # pair-swap shuffle needs no negation).
    cosf = np.repeat(fc.T, 2, axis=0).astype(np.float32)        # [HD, S]
    sinf = np.repeat(fs.T, 2, axis=0).astype(np.float32)
    sinf[0::2, :] *= -1.0
    cosf = np.ascontiguousarray(cosf)
    sinf = np.ascontiguousarray(sinf)

    onesm = np.ones((P, P), dtype=np.float32)
    ident = np.eye(P, dtype=np.float32)
    dpat_arr = (
        np.stack(plist, axis=0)
        if plist
        else np.zeros((1, P, CH), dtype=np.float32)
    )

    xtb = [np.ascontiguousarray(x[b].T) for b in range(B)]
    in_maps = []
    for c in range(N_CORES):
        b, g = divmod(c, 4)
        hcols = np.concatenate(
            [np.arange(h * HD, (h + 1) * HD) for h in (g, g + 4, g + 8, g + 12)]
        )
        in_maps.append(
            dict(
                xt=xtb[b],
                wqs=np.ascontiguousarray(wq[:, hcols]),
                wks=np.ascontiguousarray(wk[:, g * HD:(g + 1) * HD]),
                wvs=np.ascontiguousarray(wv[:, g * HD:(g + 1) * HD]),
                wos=np.ascontiguousarray(wo[hcols, :]),
                cosf=cosf,
                sinf=sinf,
                onesm=onesm,
                ident=ident,
                dpat=dpat_arr,
            )
        )

    res = run_bass_kernel_spmd(nc, in_maps, list(range(N_CORES)))

    out_full = np.empty((B, S, DIM), dtype=np.float32)
    for c in range(N_CORES):
        b, g = divmod(c, 4)
        o = res.results[c]["out"]  # [NQC, CH//4, DIM]
        for qc in range(NQC):
            r0 = qc * CH + g * (CH // 4)
            out_full[b, r0:r0 + CH // 4, :] = o[qc]
    return out_full


# revision 47
# speedup vs baseline: 1546.7861x; 1546.7861x over previous
"""Trainium2 Bass kernel for GQA attention (dense_transformer).

Sharding (8 cores): core c = (batch b = c//4, tp-rank g = c%4).
Each core computes q/k/v projections + RoPE + causal attention + partial
output projection for heads {g, g+4, g+8, g+12} (which all map to kv head
g under the reference's jnp.tile repeat), then a 4-way ReduceScatter over
the batch group combines the output projection partials; each core ends
up with a disjoint quarter of each 512-row i-chunk of the output.

Layout choices:
  - All matmuls run in float32r (tf32-class, full PE rate at N>=256).
  - Scores are computed transposed (scores^T[j, i]) so softmax probs feed
    the P@V matmul directly as the moving operand; the softmax denominator
    comes from an extra all-ones stationary matmul into a separate PSUM
    bank (partition-broadcast for free).
  - No max-subtraction in softmax: scores are O(6) sigma for this problem
    scale, exp() stays well inside fp32 range.  The additive mask is
    applied as exp(mask) multiplicative block patterns; all-zero blocks
    are skipped entirely (causality), all-pass blocks skip the multiply.
  - x is transposed on host (d-major) so every matmul contracts along
    partitions without any on-device transposes (v needs 128x128 PE
    transposes only).
  - Single fused i-chunk pipeline: proj(qc) -> rope -> attention(qc) ->
    out-proj(qc) -> ReduceScatter(qc), all sharing one 8-bank PSUM pool,
    so collectives and DMA overlap compute of the next chunk.
"""

import sys

sys.path.insert(0, "/opt/trn_rl_repo")

import numpy as np

# ---------------------------------------------------------------- constants
B = 2
S = 2048
DIM = 2048
NH = 16
NKV = 4
HD = 128          # head dim == partition count
P = 128
CH = 512          # i-chunk columns (moving dim)
NQC = S // CH     # 4 i-chunks
NJT = S // P      # 16 j-tiles
DT = DIM // P     # 16 d-tiles (contraction)
HPC = NH // 4     # 4 heads per core
NOC = DIM // CH   # 4 output column chunks
N_CORES = 8
GROUPS = [[0, 1, 2, 3], [4, 5, 6, 7]]
SCALE = 1.0 / float(np.sqrt(HD))

_prog_cache: dict = {}


def _mask_schedule(mask):
    """Per (i-chunk, j-tile) block status from the additive mask.

    Returns (sched, patterns): sched[qc] = tuple of (jt, pat_idx|None) for
    non-skipped j-tiles; patterns = list of [P, CH] f32 multiplicative
    exp(mask) blocks (transposed to [j, i] layout).
    """
    m = np.asarray(mask, dtype=np.float32).reshape(S, S)
    pats = {}
    plist = []
    sched = []
    for qc in range(NQC):
        row = []
        for jt in range(NJT):
            blk = m[qc * CH:(qc + 1) * CH, jt * P:(jt + 1) * P]
            em = np.exp(blk.astype(np.float64)).astype(np.float32)
            if np.all(em == 1.0):
                row.append((jt, None, 0))
            elif np.all(em == 0.0):
                continue
            else:
                pt = np.ascontiguousarray(em.T)  # [j=128, i=512]
                key = pt.tobytes()
                if key not in pats:
                    pats[key] = len(plist)
                    plist.append(pt)
                # leading fully-masked i-columns can be cropped from the
                # moving dim (keep >=256 so fp32r stays at full rate)
                zc = 0
                while zc < CH and not pt[:, zc].any():
                    zc += 1
                row.append((jt, pats[key], min(zc - zc % P, CH - 256)))
        if not row:
            raise ValueError(
                f"i-chunk {qc} is fully masked; softmax would be undefined"
            )
        sched.append(tuple(row))
    return tuple(sched), plist


def _build_program(sched, n_pat):
    import concourse.bacc as bacc
    import concourse.mybir as mybir
    import concourse.tile as tile

    F32 = mybir.dt.float32
    F32R = mybir.dt.float32r
    BF16 = mybir.dt.bfloat16
    FP16 = mybir.dt.float16
    AF = mybir.ActivationFunctionType

    # does any chunk's attention read k/v from a later chunk (non-causal
    # mask)?  If so the fused one-pass pipeline is invalid: fall back to
    # two phases (all projections, then attention).
    future = any(
        jt >= (qc + 1) * (CH // P)
        for qc, row in enumerate(sched)
        for (jt, _pidx, _cr) in row
    )

    nc = bacc.Bacc(None, target_bir_lowering=False, num_devices=N_CORES)

    xt = nc.declare_dram_parameter("xt", [DIM, S], F32R, isOutput=False)
    wqs = nc.declare_dram_parameter("wqs", [DIM, HPC * HD], F32R, isOutput=False)
    wks = nc.declare_dram_parameter("wks", [DIM, HD], F32R, isOutput=False)
    wvs = nc.declare_dram_parameter("wvs", [DIM, HD], F32R, isOutput=False)
    wos = nc.declare_dram_parameter("wos", [HPC * HD, DIM], F32R, isOutput=False)
    cosf = nc.declare_dram_parameter("cosf", [HD, S], F32, isOutput=False)
    sinf = nc.declare_dram_parameter("sinf", [HD, S], F32, isOutput=False)
    onesm = nc.declare_dram_parameter("onesm", [P, P], F32R, isOutput=False)
    ident = nc.declare_dram_parameter("ident", [P, P], F32R, isOutput=False)
    dpat = nc.declare_dram_parameter("dpat", [max(n_pat, 1), P, CH], FP16, isOutput=False)
    out = nc.declare_dram_parameter("out", [NJT, P // 4, DIM], FP16, isOutput=True)

    o_part = nc.dram_tensor("o_part", [S, DIM], FP16)
    rs_out = nc.dram_tensor("rs_out", [NJT, P // 4, DIM], FP16)

    shuffle_mask = [i ^ 1 for i in range(32)]

    with tile.TileContext(nc) as tc:
        with (
            tc.tile_pool(name="const", bufs=1) as constp,
            tc.tile_pool(name="wq_sb", bufs=HPC) as wqp,
            tc.tile_pool(name="wkv_sb", bufs=2) as wkvp,
            tc.tile_pool(name="wo_sb", bufs=HPC) as wop,
            tc.tile_pool(name="kT", bufs=1) as kTp,
            tc.tile_pool(name="vsb", bufs=NJT) as vsbp,
            tc.tile_pool(name="qT", bufs=(NQC if future else 2) * HPC) as qTp,
            tc.tile_pool(name="ohT", bufs=(1 if future else 2) * HPC) as ohTp,
            tc.tile_pool(name="xtp", bufs=4 if future else 8) as xtp,
            tc.tile_pool(name="vt_sb", bufs=3) as vtsbp,
            tc.tile_pool(name="probs", bufs=5) as probsp,
            tc.tile_pool(name="tmp", bufs=6) as tmpp,
            tc.tile_pool(name="o_sb", bufs=4) as osbp,
            tc.tile_pool(name="rsbp", bufs=1) as rsbp,
            tc.tile_pool(name="ps", bufs=8, space="PSUM") as psp,
        ):
            # ---- constants + resident weights -------------------------------
            cos_sb = constp.tile([HD, S], F32, tag="cos")
            sin_sb = constp.tile([HD, S], F32, tag="sin")
            ones_sb = constp.tile([P, P], F32R, tag="ones")
            id_sb = constp.tile([P, P], F32R, tag="ident")
            dpat_sb = constp.tile([P, max(n_pat, 1) * CH], FP16, tag="dpat")

            wq_sb = [
                wqp.tile([P, DT * HD], F32R, tag="wq", name=f"wq{hl}")
                for hl in range(HPC)
            ]
            wk_sb = wkvp.tile([P, DT * HD], F32R, tag="wkv")
            wv_sb = wkvp.tile([P, DT * HD], F32R, tag="wkv")
            wo_sb = [
                wop.tile([HD, DIM], F32R, tag="wo", name=f"wo{hl}")
                for hl in range(HPC)
            ]

            # warm-up loads, interleaved so the first projection matmuls and
            # the first RoPE can start as early as possible
            pre_xt = []

            def xt_load(d):
                t = xtp.tile([P, CH], F32R, tag="xt", name=f"xt0_{d}")
                nc.sync.dma_start(t[:], xt[d * P:(d + 1) * P, 0:CH])
                pre_xt.append(t)

            def wkv_quarter(db):
                DQ = DT // 4
                dbs = slice(db * DQ * HD, (db + 1) * DQ * HD)
                rows = slice(db * DQ * P, (db + 1) * DQ * P)
                nc.sync.dma_start(
                    wk_sb[:, dbs].rearrange("p (t m) -> p t m", m=HD),
                    wks[rows, :].rearrange("(t p) m -> p t m", p=P),
                )
                nc.sync.dma_start(
                    wv_sb[:, dbs].rearrange("p (t m) -> p t m", m=HD),
                    wvs[rows, :].rearrange("(t p) m -> p t m", p=P),
                )

            def wq_quarter(db):
                DQ = DT // 4
                dbs = slice(db * DQ * HD, (db + 1) * DQ * HD)
                rows = slice(db * DQ * P, (db + 1) * DQ * P)
                for hl in range(HPC):
                    nc.sync.dma_start(
                        wq_sb[hl][:, dbs].rearrange("p (t m) -> p t m", m=HD),
                        wqs[rows, hl * HD:(hl + 1) * HD].rearrange("(t p) m -> p t m", p=P),
                    )

            wkv_quarter(0)
            wq_quarter(0)
            xt_load(0)
            xt_load(1)
            wkv_quarter(1)
            wq_quarter(1)
            xt_load(2)
            xt_load(3)
            wkv_quarter(2)
            wq_quarter(2)
            xt_load(4)
            xt_load(5)
            wkv_quarter(3)
            wq_quarter(3)

            kT = kTp.tile([HD, S], F32R, tag="kT")
            vsb = [vsbp.tile([P, HD], F32R, tag="vsb", name=f"vsb{i}") for i in range(NJT)]

            def emit_oproj(qc, ohT):
                # output projection for chunk qc's rows + ReduceScatter
                for oc in range(NOC):
                    osl = slice(oc * CH, (oc + 1) * CH)
                    for it in range(CH // P):
                        ti = qc * (CH // P) + it
                        isl = slice(ti * P, (ti + 1) * P)
                        ps_o = psp.tile([P, CH], F32, tag="ps", name=f"pso{qc}_{it}_{oc}")
                        for hl in range(HPC):
                            nc.tensor.matmul(
                                ps_o[:],
                                ohT[hl][:, it * P:(it + 1) * P],
                                wo_sb[hl][:, osl],
                                start=(hl == 0), stop=(hl == HPC - 1),
                            )
                        ob = osbp.tile([P, CH], FP16, tag="ob", name=f"ob{qc}")
                        nc.scalar.activation(ob[:], ps_o[:], AF.Copy)
                        nc.sync.dma_start(o_part[isl, osl], ob[:])
                nc.gpsimd.collective_compute(
                    "ReduceScatter",
                    mybir.AluOpType.add,
                    replica_groups=GROUPS,
                    ins=[o_part[qc * CH:(qc + 1) * CH, :]],
                    outs=[rs_out[qc * (CH // P):(qc + 1) * (CH // P)]],
                )
                rsb = rsbp.tile([P, DIM], FP16, tag="rsb", name=f"rsb{qc}")
                nc.gpsimd.dma_start(
                    rsb[:],
                    rs_out[qc * (CH // P):(qc + 1) * (CH // P)].rearrange(
                        "a b c -> (a b) c"
                    ),
                )
                nc.gpsimd.dma_start(
                    out[qc * (CH // P):(qc + 1) * (CH // P)].rearrange(
                        "a b c -> (a b) c"
                    ),
                    rsb[:],
                )

            def emit_attn(qc, qT):
                # attention for all heads on this chunk
                acts = sched[qc]
                nact = len(acts)
                ohT = []
                for hl in range(HPC):
                    ps_av = psp.tile([HD, CH], F32, tag="ps", name=f"psav{qc}_{hl}")
                    ps_den = psp.tile([P, CH], F32, tag="ps", name=f"psden{qc}_{hl}")

                    def emit_avden(jt, pr, cr, idx):
                        nc.tensor.matmul(
                            ps_av[:, cr:], vsb[jt][:], pr[:, cr:],
                            start=(idx == 0), stop=(idx == nact - 1),
                        )
                        nc.tensor.matmul(
                            ps_den[:, cr:], ones_sb[:], pr[:, cr:],
                            start=(idx == 0), stop=(idx == nact - 1),
                        )

                    # software-pipelined: scores(jt+1) issues before av/den(jt)
                    # so the exp latency hides behind the next scores matmul
                    prev = None
                    for idx, (jt, pidx, crop) in enumerate(acts):
                        cr = crop if idx > 0 else 0
                        ps_s = psp.tile([P, CH], F32, tag="ps", name=f"pss{qc}_{hl}_{jt}")
                        nc.tensor.matmul(
                            ps_s[:, cr:],
                            kT[:, jt * P:(jt + 1) * P],
                            qT[hl][:, cr:],
                            start=True, stop=True,
                        )
                        pr = probsp.tile([P, CH], F32R, tag="pr", name=f"pr{qc}")
                        nc.scalar.activation(pr[:, cr:], ps_s[:, cr:], AF.Exp, scale=SCALE)
                        if pidx is not None:
                            nc.vector.tensor_mul(
                                pr[:, cr:], pr[:, cr:],
                                dpat_sb[:, pidx * CH + cr:(pidx + 1) * CH],
                            )
                        if prev is not None:
                            emit_avden(*prev)
                        prev = (jt, pr, cr, idx)
                    emit_avden(*prev)
                    inv = tmpp.tile([P, CH], F32, tag="tmp", name=f"inv{qc}")
                    nc.vector.reciprocal(inv[:], ps_den[:])
                    oh = ohTp.tile([HD, CH], F32R, tag="ohT", name=f"ohT{qc}_{hl}")
                    nc.vector.tensor_mul(oh[:], ps_av[:], inv[:])
                    ohT.append(oh)
                return ohT

            # ---- fused per-chunk pipeline -----------------------------------
            prev_ohT = None
            saved_qT = []
            for qc in range(NQC):
                csl = slice(qc * CH, (qc + 1) * CH)

                # projections: accumulate q (4 heads), k, v over d-tiles
                ps_q = [psp.tile([P, CH], F32, tag="ps", name=f"psq{qc}_{i}") for i in range(HPC)]
                ps_k = psp.tile([P, CH], F32, tag="ps", name=f"psk{qc}")
                ps_v = psp.tile([P, CH], F32, tag="ps", name=f"psv{qc}")
                for d in range(DT):
                    if qc == 0 and d < len(pre_xt):
                        xtile = pre_xt[d]
                    else:
                        xtile = xtp.tile([P, CH], F32R, tag="xt", name=f"xt{qc}_{d}")
                        nc.sync.dma_start(xtile[:], xt[d * P:(d + 1) * P, csl])
                    dsl = slice(d * HD, (d + 1) * HD)
                    st = (d == 0)
                    sp = (d == DT - 1)
                    nc.tensor.matmul(ps_k[:], wk_sb[:, dsl], xtile[:], start=st, stop=sp)
                    nc.tensor.matmul(ps_v[:], wv_sb[:, dsl], xtile[:], start=st, stop=sp)
                    for hl in range(HPC):
                        nc.tensor.matmul(
                            ps_q[hl][:], wq_sb[hl][:, dsl], xtile[:],
                            start=st, stop=sp,
                        )

                # per-chunk slices of the RoPE tables: chunk 0's rope only
                # needs the first 512 columns, so don't front-load 4MB
                nc.sync.dma_start(cos_sb[:, csl], cosf[:, csl])
                nc.sync.dma_start(sin_sb[:, csl], sinf[:, csl])
                if qc == 0:
                    nc.sync.dma_start(ones_sb[:], onesm[:])
                    nc.sync.dma_start(id_sb[:], ident[:])
                    nc.sync.dma_start(
                        dpat_sb[:].rearrange("p (n c) -> p n c", c=CH),
                        dpat[:].rearrange("n p c -> p n c"),
                    )
                    for oc in range(NOC):
                        for hl in range(HPC):
                            nc.sync.dma_start(
                                wo_sb[hl][:, oc * CH:(oc + 1) * CH],
                                wos[hl * HD:(hl + 1) * HD, oc * CH:(oc + 1) * CH],
                            )

                # RoPE q heads -> per-chunk qT tiles; k -> resident kT
                qT = []
                for hl in range(HPC):
                    dst = qTp.tile([HD, CH], F32R, tag="qT", name=f"qT{qc}_{hl}")
                    qT.append(dst)
                # q heads first: attention on this chunk is gated by qT (old kT
                # columns are already rope'd); the new kT columns are only
                # needed at the diagonal j-tiles, late in the j-loop.
                rope_jobs = [(ps_q[hl], qT[hl][:]) for hl in range(HPC)]
                rope_jobs.append((ps_k, kT[:, csl]))
                if qc == 0:
                    rope_jobs.insert(0, rope_jobs.pop())
                for src, dst in rope_jobs:
                    swp = tmpp.tile([P, CH], F32, tag="tmp", name=f"sw{qc}")
                    nc.vector.stream_shuffle(swp[:], src[:], shuffle_mask)
                    tcos = tmpp.tile([P, CH], F32, tag="tmp", name=f"tc{qc}")
                    nc.vector.tensor_mul(tcos[:], src[:], cos_sb[:, csl])
                    tsin = tmpp.tile([P, CH], F32, tag="tmp", name=f"ts{qc}")
                    nc.vector.tensor_mul(tsin[:], swp[:], sin_sb[:, csl])
                    nc.vector.tensor_add(dst, tcos[:], tsin[:])

                # v: copy out of psum and transpose into [j, c] tiles
                vt = vtsbp.tile([P, CH], F32R, tag="vt", name=f"vt{qc}")
                nc.scalar.activation(vt[:], ps_v[:], AF.Copy)
                for jl in range(CH // P):
                    tps = psp.tile([P, P], F32R, tag="ps", name=f"tps{qc}_{jl}")
                    nc.tensor.transpose(tps[:], vt[:, jl * P:(jl + 1) * P], id_sb[:])
                    nc.scalar.activation(vsb[qc * (CH // P) + jl][:], tps[:], AF.Copy)

                if future:
                    saved_qT.append(qT)
                    continue

                # previous chunk's output projection: PE work that overlaps the
                # RoPE (DVE) of this chunk
                if prev_ohT is not None:
                    emit_oproj(qc - 1, prev_ohT)
                prev_ohT = emit_attn(qc, qT)

            if future:
                for qc in range(NQC):
                    emit_oproj(qc, emit_attn(qc, saved_qT[qc]))
            else:
                emit_oproj(NQC - 1, prev_ohT)

    nc.finalize()
    return nc


def _get_program(sched, n_pat):
    key = (sched, n_pat)
    if key not in _prog_cache:
        _prog_cache[key] = _build_program(sched, n_pat)
    return _prog_cache[key]


def kernel(x, wq, wk, wv, wo, freqs_cos, freqs_sin, mask, start_pos=0, **_kw):
    from concourse.bass_utils import run_bass_kernel_spmd

    x = np.asarray(x, dtype=np.float32)
    wq = np.asarray(wq, dtype=np.float32)
    wk = np.asarray(wk, dtype=np.float32)
    wv = np.asarray(wv, dtype=np.float32)
    wo = np.asarray(wo, dtype=np.float32)
    fc = np.asarray(freqs_cos, dtype=np.float32)
    fs = np.asarray(freqs_sin, dtype=np.float32)

    sched, plist = _mask_schedule(mask)
    nc = _get_program(sched, len(plist))

    # RoPE tables expanded to head-dim channels (sin sign-interleaved so the
    # pair-swap shuffle needs no negation).
    cosf = np.repeat(fc.T, 2, axis=0).astype(np.float32)        # [HD, S]
    sinf = np.repeat(fs.T, 2, axis=0).astype(np.float32)
    sinf[0::2, :] *= -1.0
    cosf = np.ascontiguousarray(cosf)
    sinf = np.ascontiguousarray(sinf)

    onesm = np.ones((P, P), dtype=np.float32)
    ident = np.eye(P, dtype=np.float32)
    dpat_arr = (
        np.stack(plist, axis=0).astype(np.float16)
        if plist
        else np.zeros((1, P, CH), dtype=np.float16)
    )

    xtb = [np.ascontiguousarray(x[b].T) for b in range(B)]
    in_maps = []
    for c in range(N_CORES):
        b, g = divmod(c, 4)
        hcols = np.concatenate(
            [np.arange(h * HD, (h + 1) * HD) for h in (g, g + 4, g + 8, g + 12)]
        )
        in_maps.append(
            dict(
                xt=xtb[b],
                wqs=np.ascontiguousarray(wq[:, hcols]),
                wks=np.ascontiguousarray(wk[:, g * HD:(g + 1) * HD]),
                wvs=np.ascontiguousarray(wv[:, g * HD:(g + 1) * HD]),
                wos=np.ascontiguousarray(wo[hcols, :]),
                cosf=cosf,
                sinf=sinf,
                onesm=onesm,
                ident=ident,
                dpat=dpat_arr,
            )
        )

    res = run_bass_kernel_spmd(nc, in_maps, list(range(N_CORES)))

    out_full = np.empty((B, S, DIM), dtype=np.float32)
    for c in range(N_CORES):
        b, g = divmod(c, 4)
        o = np.asarray(res.results[c]["out"], dtype=np.float32)
        for qc in range(NQC):
            r0 = qc * CH + g * P
            out_full[b, r0:r0 + P, :] = o[qc * (CH // P):(qc + 1) * (CH // P)].reshape(P, DIM)
    return out_full
